# revision 49
# baseline (speedup 1.0000x reference)
# Trainium2 Bass kernel for nn_AttentionStream (dense transformer block with
# relative-position attention), SPMD over 8 NeuronCores.
#
# Sharding: core c -> batch b = c//2, head-group hg = c%2 (4 heads each).
# Each core computes a row-parallel partial of the output projection for its
# batch; the host sums the two partials per batch and adds the bias.
#
# Device algorithm (per core), "transposed flash" layout (PV needs no
# transposes), with the positional term folded in as LOGIT ADDS done on the
# TENSOR engine (identity-matmul accumulation into the dots PSUM group, so
# the exp never waits on another engine):
#   qT/kT = projections of x (d on partitions), v in [r, d] layout
#   L[n, j] = q~ . relF[j]   (relF host-prepped: reversed, minus
#       rel[dist=far-past] so left-clamp add == 0, edge-padded) -> DRAM bf16
#   per (head, n-window): dots^T[r, n] tiles in PSUM, then in the same
#       accumulation group:
#         band tiles:   += Id.T @ Lskew   (skew via merged transpose-DMA)
#         right tiles:  += ones_row.T @ edelL[n]  (K=1 rank-1 broadcast)
#       single exp (ACT) -> P bf16
#   PV: acc[65, n] += [v | 1].T @ P   (ones row accumulates the denominator)
#   avn = acc[0:64] / acc[64] ; out^T += Wo_hpair.T @ avn (head-pair packed)
import os
import sys

import numpy as np
import ml_dtypes

for _p in ("/opt/trn_rl_repo", "/root/.axon_site/_ro/trn_rl_repo"):
    if _p not in sys.path and os.path.isdir(_p):
        sys.path.append(_p)

B, N, DIM = 4, 2048, 512
H, D = 8, 64          # total heads, head dim
HPC = 4               # heads per core
INNER = H * D
MAXP = 512
SCALE = D ** -0.5
NCORES = 8
W = 1280              # padded j width; j' = j + PAD_L, j in [-128, 1151]
PAD_L = 128
NW = 8                # n-windows of 256
NRC = 16              # r-chunks of 128

BF = ml_dtypes.bfloat16

_CACHE = {}


def _build_bass():
    import concourse.bass as bass
    import concourse.mybir as mybir
    import concourse.tile as tile
    from concourse import bacc

    dt = mybir.dt
    fp32 = dt.float32
    bf16 = dt.bfloat16
    EXP = mybir.ActivationFunctionType.Exp
    LOG_SCALE = -5.545177444479562  # -ln(256): scales P by 2^-8 so the
    # denominator fits fp16 for the reciprocal-broadcast (ratio invariant)

    nc = bacc.Bacc("TRN2", target_bir_lowering=False, debug=False,
                   num_devices=NCORES)

    xT = nc.dram_tensor("xT", [DIM, N], bf16, kind="ExternalInput")
    wq = nc.dram_tensor("wq", [DIM, 256], bf16, kind="ExternalInput")
    wk = nc.dram_tensor("wk", [DIM, 256], bf16, kind="ExternalInput")
    wv = nc.dram_tensor("wv", [DIM, 256], bf16, kind="ExternalInput")
    relT = nc.dram_tensor("relT", [128, W], bf16, kind="ExternalInput")
    wo = nc.dram_tensor("wo", [256, DIM], bf16, kind="ExternalInput")
    edel = nc.dram_tensor("edel", [HPC, N], bf16, kind="ExternalInput")
    outT = nc.dram_tensor("outT", [DIM, N], fp32, kind="ExternalOutput")

    from contextlib import ExitStack
    with tile.TileContext(nc) as tc, ExitStack() as ctx:
        consts = ctx.enter_context(tc.tile_pool(name="consts", bufs=1))
        lpool = ctx.enter_context(tc.tile_pool(name="lpool", bufs=6))
        work = ctx.enter_context(tc.tile_pool(name="work", bufs=4))
        ppool = ctx.enter_context(tc.tile_pool(name="ppool", bufs=3))
        eppool = ctx.enter_context(tc.tile_pool(name="eppool", bufs=2))
        numep = ctx.enter_context(tc.tile_pool(name="numep", bufs=3))
        psb = ctx.enter_context(tc.tile_pool(name="psb", bufs=3, space="PSUM"))
        psa = ctx.enter_context(tc.tile_pool(name="psa", bufs=2, space="PSUM"))
        dramp = ctx.enter_context(tc.tile_pool(name="dramp", bufs=4, space="DRAM"))

        # ---- load constants (scalar-engine HWDGE queue) ---------------------
        xT_sb = consts.tile([128, 4, N], bf16, tag="xT_sb")
        nc.scalar.dma_start(out=xT_sb, in_=xT.ap().rearrange("(c p) n -> p c n", p=128))
        wq_sb = consts.tile([128, 4, 256], bf16, tag="wq_sb")
        nc.scalar.dma_start(out=wq_sb, in_=wq.ap().rearrange("(c p) i -> p c i", p=128))
        wk_sb = consts.tile([128, 4, 256], bf16, tag="wk_sb")
        nc.scalar.dma_start(out=wk_sb, in_=wk.ap().rearrange("(c p) i -> p c i", p=128))
        wv_sb = consts.tile([128, 4, 256], bf16, tag="wv_sb")
        nc.scalar.dma_start(out=wv_sb, in_=wv.ap().rearrange("(c p) i -> p c i", p=128))
        relT_sb = consts.tile([128, W], bf16, tag="relT_sb")
        nc.scalar.dma_start(out=relT_sb, in_=relT.ap())
        # wo packed as head-pairs: rows (hc*128 + hp*64 + d) -> [128, 2, DIM]
        wo_sb = consts.tile([128, 2, DIM], bf16, tag="wo_sb")
        nc.scalar.dma_start(out=wo_sb, in_=wo.ap().rearrange("(c p) o -> p c o", p=128))
        # edel factors exp(delta) replicated across all 128 partitions
        edel_sb = consts.tile([128, HPC, N], bf16, tag="edel_sb")
        edel_src = bass.AP(tensor=edel.ap().tensor, offset=edel.ap().offset,
                           ap=[[0, 128], [N, HPC], [1, N]])
        nc.scalar.dma_start(out=edel_sb, in_=edel_src)
        # all-ones rows; [64:65, 0:64] is the K=1 lhsT that broadcasts the
        # denominator reciprocal from partition 64 to partitions 0..63
        ones_sb = consts.tile([128, 64], bf16, tag="ones_sb")
        nc.vector.memset(ones_sb, 1.0)

        # ---- projections + L-logit production, software-pipelined -----------
        # L(0)/L(1) need only the ic=0 half of qT, so emit proj-q ic0 first,
        # then stream L(0)/L(1) chunks interleaved with the remaining
        # projection work.
        qT_sb = consts.tile([128, 2, N], bf16, tag="qT_sb")
        kT_sb = consts.tile([128, 2, N], bf16, tag="kT_sb")
        v_sb = consts.tile([128, NRC, HPC, 65], bf16, tag="v_sb")
        nc.vector.memset(v_sb[:, :, :, 64], 1.0)

        def emit_qk_unit(dst_sb, w_sb, ic, nw):
            ps = psb.tile([128, 1024], fp32, tag="big")
            for dc in range(4):
                nc.tensor.matmul(
                    ps[:, 0:512],
                    lhsT=w_sb[:, dc, ic * 128:(ic + 1) * 128],
                    rhs=xT_sb[:, dc, nw * 512:(nw + 1) * 512],
                    start=(dc == 0), stop=(dc == 3))
            nc.vector.tensor_copy(dst_sb[:, ic, nw * 512:(nw + 1) * 512],
                                  ps[:, 0:512])

        def emit_v_unit(rc):
            ps = psb.tile([128, 1024], fp32, tag="big")
            for dc in range(4):
                nc.tensor.matmul(
                    ps[:, 0:256],
                    lhsT=xT_sb[:, dc, rc * 128:(rc + 1) * 128],
                    rhs=wv_sb[:, dc, :],
                    start=(dc == 0), stop=(dc == 3))
            nc.vector.tensor_copy(
                v_sb[:, rc, :, 0:64],
                ps[:, 0:256].rearrange("p (h d) -> p h d", h=HPC))

        L_dram_h = [None] * HPC
        for h in range(HPC):
            L_dram_h[h] = dramp.tile([N, W], bf16, tag="Ldram",
                                     name=f"Ldram{h}")

        # L chunks are emitted in pairs sharing one tail PSUM tile
        def emit_L_pair(chunks, cast_on_act=False):
            tail = psb.tile([128, 1024], fp32, tag="big")
            for idx, (h, nck) in enumerate(chunks):
                hc, hp = h // 2, (h % 2) * 64
                lsb = lpool.tile([128, W], bf16, tag="lsb")
                psA = psb.tile([128, 1024], fp32, tag="big")
                for jw in range(2):
                    nc.tensor.matmul(
                        psA[:, jw * 512:(jw + 1) * 512],
                        lhsT=qT_sb[hp:hp + 64, hc, nck * 128:(nck + 1) * 128],
                        rhs=relT_sb[hp:hp + 64, jw * 512:(jw + 1) * 512],
                        start=True, stop=True)
                nc.tensor.matmul(
                    tail[:, idx * 512:idx * 512 + 256],
                    lhsT=qT_sb[hp:hp + 64, hc, nck * 128:(nck + 1) * 128],
                    rhs=relT_sb[hp:hp + 64, 1024:1280],
                    start=True, stop=True)
                if cast_on_act:
                    nc.scalar.copy(lsb[:, 0:1024], psA)
                else:
                    nc.vector.tensor_copy(lsb[:, 0:1024], psA)
                nc.vector.tensor_copy(lsb[:, 1024:1280],
                                      tail[:, idx * 512:idx * 512 + 256])
                nc.gpsimd.dma_start(
                    out=L_dram_h[h][nck * 128:(nck + 1) * 128, :], in_=lsb)

        ep_all_h = [None] * HPC

        def emit_transposes(h):
            """Merged skew transpose-DMAs: one per r-chunk, covering all its
            band n-subchunks (diagonals d = rc - s in [-4, 4])."""
            L_dram = L_dram_h[h]
            ep_all = eppool.tile([128, NRC, 9, 128], bf16, tag="ep_all",
                                 name=f"ep_all{h}")
            ep_all_h[h] = ep_all
            for rc in range(NRC):
                s_lo, s_hi = max(0, rc - 4), min(NRC - 1, rc + 4)
                k = s_hi - s_lo + 1
                # src element (s, a, c) = L[128*s + c, PAD + 128*(rc-s) + 512 + a - c]
                off = (L_dram.offset + 128 * s_lo * W
                       + PAD_L + 128 * (rc - s_lo) + 512)
                src = bass.AP(tensor=L_dram.tensor, offset=off,
                              ap=[[W - 1, 128 * k], [1, 128]])
                slot0 = s_lo - (rc - 4)
                nc.sync.dma_start(out=ep_all[:, rc, slot0:slot0 + k, :],
                                  in_=src, transpose=True)

        def emit_unit(u):
            if u[0] == "qk":
                emit_qk_unit(u[1], u[2], u[3], u[4])
            else:
                emit_v_unit(u[1])

        for nw in range(4):
            emit_qk_unit(qT_sb, wq_sb, 0, nw)
        # startup minimal set: k-ic0 (att(0) dots) + v (att(0) PV) + L(0)
        unit_q = [("qk", kT_sb, wk_sb, 0, nw) for nw in range(4)] \
            + [("v", rc) for rc in range(NRC)]
        ri = 0
        for pr in range(8):
            emit_L_pair([(0, 2 * pr), (0, 2 * pr + 1)], cast_on_act=True)
            for _ in range(2):
                if ri < len(unit_q):
                    emit_unit(unit_q[ri]); ri += 1
        while ri < len(unit_q):
            emit_unit(unit_q[ri]); ri += 1
        emit_transposes(0)

        avn_all = consts.tile([128, 2, NW, 256], bf16, tag="avn_all")

        def emit_att(h, extras=None, next_transposes=None):
            hc, hp = h // 2, (h % 2) * 64
            extras = extras or {}
            num_all = numep.tile([65, NW, 256], bf16, tag="num_all")
            ep_all = ep_all_h[h]

            deferred = []   # emissions delayed into the next window

            for nw in range(NW):
                n0 = nw * 256
                s0 = 2 * nw                     # first n-sub of this window
                P_sb = ppool.tile([128, NRC, 256], bf16, tag="P_sb")
                acc = psa.tile([128, 256], fp32, tag="acc")
                ca, cb = max(0, s0 - 3), min(NRC - 1, s0 + 4)

                def emit_group(g, ep_all=ep_all, n0=n0, s0=s0, ca=ca, cb=cb,
                               P_sb=P_sb):
                    ps = psb.tile([128, 1024], fp32, tag="big")
                    for i in range(4):
                        rc = 4 * g + i
                        nc.tensor.matmul(
                            ps[:, i * 256:(i + 1) * 256],
                            lhsT=kT_sb[hp:hp + 64, hc, rc * 128:(rc + 1) * 128],
                            rhs=qT_sb[hp:hp + 64, hc, n0:n0 + 256],
                            start=True, stop=True)
                    psv = ps.rearrange("p (i n) -> p i n", i=4)
                    # pre-exp band-logit adds restricted to this group's rc's
                    a, b = max(ca, 4 * g), min(cb, 4 * g + 3)
                    if b >= a:
                        kk = b - a + 1
                        epa = ep_all[:, a, s0 - a + 4, 0:128]   # first element
                        ep_run = bass.AP(
                            tensor=epa.tensor, offset=epa.offset,
                            ap=[list(epa.ap[0]), [8 * 128, kk], [1, 256]])
                        nc.vector.tensor_add(psv[:, a - 4 * g:b - 4 * g + 1, :],
                                             psv[:, a - 4 * g:b - 4 * g + 1, :],
                                             ep_run)
                    rc = s0 - 4
                    if 4 * g <= rc <= 4 * g + 3:
                        c0 = (rc - 4 * g) * 256
                        nc.vector.tensor_add(ps[:, c0:c0 + 128],
                                             ps[:, c0:c0 + 128],
                                             ep_all[:, rc, 8, :])
                    rc = s0 + 5
                    if 4 * g <= rc <= 4 * g + 3 and rc <= NRC - 1:
                        c0 = (rc - 4 * g) * 256 + 128
                        nc.vector.tensor_add(ps[:, c0:c0 + 128],
                                             ps[:, c0:c0 + 128],
                                             ep_all[:, rc, 0, :])
                    nc.scalar.activation(
                        P_sb[:, 4 * g:4 * (g + 1), :], psv, EXP,
                        bias=bias_sb)

                def emit_pv(rcs):
                    for rc in rcs:
                        nc.tensor.matmul(
                            acc[0:65, :],
                            lhsT=v_sb[:, rc, h, :],
                            rhs=P_sb[:, rc, :],
                            start=(rc == 0), stop=(rc == NRC - 1),
                            skip_group_check=True)

                emit_group(0)
                emit_group(1)
                emit_group(2)
                emit_group(3)
                if deferred:
                    deferred[0]()
                for th_x in extras.get(nw, ()):
                    th_x()
                if deferred:
                    deferred[1]()
                deferred = []

                def make_tail(P_sb=P_sb, acc=acc, nw=nw, n0=n0, s0=s0):
                    right = list(range(min(s0 + 6, NRC), NRC))
                    band = list(range(0, min(s0 + 6, NRC)))

                    def t1():
                        # right-clamp region first, then scale the partial
                        # accumulator by exp(delta)[n] in one [65,256] op
                        for i, rc in enumerate(right):
                            nc.tensor.matmul(
                                acc[0:65, :],
                                lhsT=v_sb[:, rc, h, :],
                                rhs=P_sb[:, rc, :],
                                start=(i == 0), stop=False,
                                skip_group_check=True)
                        if right:
                            nc.vector.tensor_mul(
                                acc[0:65, :], acc[0:65, :],
                                edel_sb[0:65, h, n0:n0 + 256])
                        rc5 = s0 + 5
                        if rc5 <= NRC - 1:
                            # mixed tile: first half is right-clamp
                            nc.vector.tensor_mul(
                                P_sb[:, rc5, 0:128], P_sb[:, rc5, 0:128],
                                edel_sb[:, h, n0:n0 + 128])

                    def t2():
                        for i, rc in enumerate(band):
                            nc.tensor.matmul(
                                acc[0:65, :],
                                lhsT=v_sb[:, rc, h, :],
                                rhs=P_sb[:, rc, :],
                                start=(not right and i == 0),
                                stop=(rc == band[-1]),
                                skip_group_check=True)
                        nc.vector.tensor_copy(num_all[:, nw, :], acc[0:65, :])
                        # per-window softmax division: 1/den on partition 64,
                        # matmul-broadcast down to partitions 0..63, multiply
                        r_tile = work.tile([128, 256], fp32, tag="r_tile")
                        with nc.allow_low_precision(
                                reason="fp16 1/den; den scaled to ~2^4"):
                            nc.vector.reciprocal(r_tile[64:65, :],
                                                 acc[64:65, :])
                        bc = psa.tile([128, 256], fp32, tag="acc",
                                      name="bc")
                        nc.tensor.matmul(
                            bc[0:64, :],
                            lhsT=ones_sb[64:65, :],
                            rhs=r_tile[64:65, :],
                            start=True, stop=True)
                        if hp == 0:
                            nc.vector.tensor_mul(avn_all[0:64, hc, nw, :],
                                                 num_all[0:64, nw, :],
                                                 bc[0:64, :])
                        else:
                            # odd head lives on partitions 64-127; DVE can't
                            # shift partitions, so hop through a DMA
                            at = work.tile([64, 256], bf16, tag="avn_tmp")
                            nc.vector.tensor_mul(at, num_all[0:64, nw, :],
                                                 bc[0:64, :])
                            nc.gpsimd.dma_start(
                                out=avn_all[64:128, hc, nw, :], in_=at)
                    return [t1, t2]
                deferred = make_tail()

            if deferred:
                deferred[0]()
                deferred[1]()
            if next_transposes is not None:
                emit_transposes(next_transposes)

        def emit_out_unit(nw):
            # output projection for one n-window (head-pair packed matmuls)
            for oc in range(4):
                ps = psb.tile([128, 1024], fp32, tag="big")
                for hcc in range(2):
                    nc.tensor.matmul(
                        ps[:, oc * 256:(oc + 1) * 256],
                        lhsT=wo_sb[:, hcc, oc * 128:(oc + 1) * 128],
                        rhs=avn_all[:, hcc, nw, :],
                        start=(hcc == 0), stop=(hcc == 1))
                o_sb = work.tile([128, 256], fp32, tag="o_sb")
                nc.vector.tensor_copy(o_sb, ps[:, oc * 256:(oc + 1) * 256])
                nc.gpsimd.dma_start(
                    out=outT.ap()[oc * 128:(oc + 1) * 128,
                                  nw * 256:(nw + 1) * 256],
                    in_=o_sb)

        # heads pipeline: startup produced qT/kT-ic0/v/L(0); everything else
        # streams inside the att window loops.
        def lp(h, pr):
            return lambda: emit_L_pair([(h, 2 * pr), (h, 2 * pr + 1)])

        def un(u):
            return lambda: emit_unit(u)

        def tp(h):
            return lambda: emit_transposes(h)

        uq1 = [("qk", qT_sb, wq_sb, 1, nw) for nw in range(4)]
        uk1 = [("qk", kT_sb, wk_sb, 1, nw) for nw in range(4)]
        emit_att(0, extras={
            0: [lp(1, 0), lp(1, 1)],
            1: [lp(1, 2), lp(1, 3)],
            2: [lp(1, 4), lp(1, 5)],
            3: [lp(1, 6), lp(1, 7)],
            4: [tp(1)],
            5: [un(uq1[0]), un(uq1[1])],
            6: [un(uq1[2]), un(uq1[3]), un(uk1[0])],
            7: [un(uk1[1]), un(uk1[2]), un(uk1[3])],
        })
        emit_att(1, extras={
            0: [lp(2, 0), lp(2, 1)], 1: [lp(2, 2), lp(2, 3)],
            2: [lp(2, 4), lp(2, 5)], 3: [lp(2, 6), lp(2, 7)],
            4: [tp(2)],
        })
        emit_att(2, extras={
            0: [lp(3, 0), lp(3, 1)], 1: [lp(3, 2), lp(3, 3)],
            2: [lp(3, 4), lp(3, 5)], 3: [lp(3, 6), lp(3, 7)],
            4: [tp(3)],
        })
        emit_att(3, extras={w: [lambda w=w: emit_out_unit(w - 2)]
                            for w in range(2, NW)})

        # ---- output projection: last two windows ---------------------------
        emit_out_unit(NW - 2)
        emit_out_unit(NW - 1)

    nc.compile()
    return nc


def host_prep(x, Wq, Wkv, Wo, bo, rel_emb):
    """Build the 8 per-core input maps (all host-side prep is O(N*D))."""
    x = np.asarray(x, np.float32)
    Wq = np.asarray(Wq, np.float32)
    Wkv = np.asarray(Wkv, np.float32)
    Wo = np.asarray(Wo, np.float32)
    rel_emb = np.asarray(rel_emb, np.float32)

    # relF[j] = rel_emb[1024-j] - rel_emb[1024], edge-padded/clipped; [W, 64]
    jgrid = np.clip(np.arange(W) - PAD_L, 0, 1024)
    relF = rel_emb[1024 - jgrid] - rel_emb[1024]
    relT_one = np.ascontiguousarray(relF.T)            # [64, W]
    relT_in = np.concatenate([relT_one, relT_one], axis=0).astype(BF)  # [128, W]
    d_vec = rel_emb[0] - rel_emb[1024]                 # [64]

    in_maps = []
    for core in range(NCORES):
        b, hg = core // 2, core % 2
        sl = slice(hg * 256, (hg + 1) * 256)
        wq_s = (Wq[:, sl] * SCALE).astype(BF)
        # edelta per head (LOGITS): x @ Wq~_h @ d_vec
        delta = x[b] @ ((Wq[:, sl] * SCALE).reshape(DIM, HPC, D) @ d_vec)  # [N, HPC]
        in_maps.append({
            "xT": np.ascontiguousarray(x[b].T).astype(BF),
            "wq": wq_s,
            "wk": Wkv[:, sl].astype(BF),
            "wv": Wkv[:, 512 + hg * 256: 512 + (hg + 1) * 256].astype(BF),
            "relT": relT_in,
            "wo": Wo[sl, :].astype(BF),
            "edel": np.ascontiguousarray(np.exp(delta).T).astype(BF),  # [HPC, N]
        })
    return in_maps


def _install_ntff_hook():
    """The agent image's antenv lacks axon_hooks; synthesize it so
    run_bass_kernel_spmd(trace=True) can capture NTFF profiles."""
    import types
    try:
        if "antenv.axon_hooks" not in sys.modules:
            import antenv
            from trn_agent_boot.trn_boot import _ntff_profile_via_ctypes
            hooks = types.ModuleType("antenv.axon_hooks")
            state = {"h": _ntff_profile_via_ctypes("/opt/axon/libaxon_pjrt.so")}
            hooks.set_axon_ntff_profile_hook = lambda h: state.__setitem__("h", h)
            hooks.get_axon_ntff_profile_hook = lambda: state["h"]
            sys.modules["antenv.axon_hooks"] = hooks
            antenv.axon_hooks = hooks
        import antenv.axon_hooks as ah
        return ah.get_axon_ntff_profile_hook() is not None
    except Exception as e:
        print(f"ntff hook install failed: {e!r}")
        return False


def kernel(x, Wq, Wkv, Wo, bo, rel_emb, _trace=False):
    import concourse.bass_utils as bu
    from concourse.bass_utils import run_bass_kernel_spmd

    if "nc" not in _CACHE:
        _CACHE["nc"] = _build_bass()
    nc = _CACHE["nc"]

    in_maps = host_prep(x, Wq, Wkv, Wo, bo, rel_emb)
    kw = {}
    if _trace and _install_ntff_hook():
        bu.upload_artifacts = lambda d: d     # zero-egress: keep artifacts local
        tmpdir = "/root/problem/traces/latest"
        import shutil
        shutil.rmtree(tmpdir, ignore_errors=True)
        os.makedirs(tmpdir, exist_ok=True)
        kw = dict(trace=True, tmpdir=tmpdir)
    res = run_bass_kernel_spmd(nc, in_maps, list(range(NCORES)), **kw)
    _CACHE["last_result"] = res

    bo = np.asarray(bo, np.float32)
    out = np.empty((B, N, DIM), np.float32)
    for b in range(B):
        pT = res.results[2 * b]["outT"] + res.results[2 * b + 1]["outT"]
        out[b] = pT.T + bo[None, :]
    return out


# revision 50
# speedup vs baseline: 1.1064x; 1.1064x over previous
# Trainium2 Bass kernel for nn_AttentionStream (dense transformer block with
# relative-position attention), SPMD over 8 NeuronCores.
#
# Sharding: core c -> batch b = c//2, head-group hg = c%2 (4 heads each).
# Each core computes a row-parallel partial of the output projection for its
# batch; the host sums the two partials per batch and adds the bias.
#
# Device algorithm (per core), "transposed flash" layout (PV needs no
# transposes), with the positional term folded in as LOGIT ADDS done on the
# TENSOR engine (identity-matmul accumulation into the dots PSUM group, so
# the exp never waits on another engine):
#   qT/kT = projections of x (d on partitions), v in [r, d] layout
#   L[n, j] = q~ . relF[j]   (relF host-prepped: reversed, minus
#       rel[dist=far-past] so left-clamp add == 0, edge-padded) -> DRAM bf16
#   per (head, n-window): dots^T[r, n] tiles in PSUM, then in the same
#       accumulation group:
#         band tiles:   += Id.T @ Lskew   (skew via merged transpose-DMA)
#         right tiles:  += ones_row.T @ edelL[n]  (K=1 rank-1 broadcast)
#       single exp (ACT) -> P bf16
#   PV: acc[65, n] += [v | 1].T @ P   (ones row accumulates the denominator)
#   avn = acc[0:64] / acc[64] ; out^T += Wo_hpair.T @ avn (head-pair packed)
import os
import sys

import numpy as np
import ml_dtypes

for _p in ("/opt/trn_rl_repo", "/root/.axon_site/_ro/trn_rl_repo"):
    if _p not in sys.path and os.path.isdir(_p):
        sys.path.append(_p)

B, N, DIM = 4, 2048, 512
H, D = 8, 64          # total heads, head dim
HPC = 4               # heads per core
INNER = H * D
MAXP = 512
SCALE = D ** -0.5
NCORES = 8
W = 1280              # padded j width; j' = j + PAD_L, j in [-128, 1151]
PAD_L = 128
NW = 8                # n-windows of 256
NRC = 16              # r-chunks of 128

BF = ml_dtypes.bfloat16

_CACHE = {}


def _build_bass():
    import concourse.bass as bass
    import concourse.mybir as mybir
    import concourse.tile as tile
    from concourse import bacc

    dt = mybir.dt
    fp32 = dt.float32
    bf16 = dt.bfloat16
    EXP = mybir.ActivationFunctionType.Exp
    LOG_SCALE = -5.545177444479562  # -ln(256): scales P by 2^-8 so the
    # denominator fits fp16 for the reciprocal-broadcast (ratio invariant)

    nc = bacc.Bacc("TRN2", target_bir_lowering=False, debug=False,
                   num_devices=NCORES)

    xT = nc.dram_tensor("xT", [DIM, N], bf16, kind="ExternalInput")
    wq = nc.dram_tensor("wq", [DIM, 256], bf16, kind="ExternalInput")
    wk = nc.dram_tensor("wk", [DIM, 256], bf16, kind="ExternalInput")
    wv = nc.dram_tensor("wv", [DIM, 256], bf16, kind="ExternalInput")
    relT = nc.dram_tensor("relT", [128, W], bf16, kind="ExternalInput")
    wo = nc.dram_tensor("wo", [256, DIM], bf16, kind="ExternalInput")
    edel = nc.dram_tensor("edel", [HPC, N], bf16, kind="ExternalInput")
    outT = nc.dram_tensor("outT", [DIM, N], fp32, kind="ExternalOutput")

    from contextlib import ExitStack
    with tile.TileContext(nc) as tc, ExitStack() as ctx:
        consts = ctx.enter_context(tc.tile_pool(name="consts", bufs=1))
        lpool = ctx.enter_context(tc.tile_pool(name="lpool", bufs=6))
        work = ctx.enter_context(tc.tile_pool(name="work", bufs=4))
        ppool = ctx.enter_context(tc.tile_pool(name="ppool", bufs=2))
        eppool = ctx.enter_context(tc.tile_pool(name="eppool", bufs=2))
        numep = ctx.enter_context(tc.tile_pool(name="numep", bufs=2))
        psb = ctx.enter_context(tc.tile_pool(name="psb", bufs=3, space="PSUM"))
        psa = ctx.enter_context(tc.tile_pool(name="psa", bufs=2, space="PSUM"))
        dramp = ctx.enter_context(tc.tile_pool(name="dramp", bufs=4, space="DRAM"))

        # ---- load constants (scalar-engine HWDGE queue) ---------------------
        xT_sb = consts.tile([128, 4, N], bf16, tag="xT_sb")
        nc.scalar.dma_start(out=xT_sb, in_=xT.ap().rearrange("(c p) n -> p c n", p=128))
        wq_sb = consts.tile([128, 4, 256], bf16, tag="wq_sb")
        nc.scalar.dma_start(out=wq_sb, in_=wq.ap().rearrange("(c p) i -> p c i", p=128))
        wk_sb = consts.tile([128, 4, 256], bf16, tag="wk_sb")
        nc.scalar.dma_start(out=wk_sb, in_=wk.ap().rearrange("(c p) i -> p c i", p=128))
        wv_sb = consts.tile([128, 4, 256], bf16, tag="wv_sb")
        nc.scalar.dma_start(out=wv_sb, in_=wv.ap().rearrange("(c p) i -> p c i", p=128))
        relT_sb = consts.tile([128, W], bf16, tag="relT_sb")
        nc.scalar.dma_start(out=relT_sb, in_=relT.ap())
        # wo packed as head-pairs: rows (hc*128 + hp*64 + d) -> [128, 2, DIM]
        wo_sb = consts.tile([128, 2, DIM], bf16, tag="wo_sb")
        nc.scalar.dma_start(out=wo_sb, in_=wo.ap().rearrange("(c p) o -> p c o", p=128))
        # edel factors exp(delta) replicated across all 128 partitions
        edel_sb = consts.tile([128, HPC, N], bf16, tag="edel_sb")
        edel_src = bass.AP(tensor=edel.ap().tensor, offset=edel.ap().offset,
                           ap=[[0, 128], [N, HPC], [1, N]])
        nc.scalar.dma_start(out=edel_sb, in_=edel_src)
        # all-ones rows; [64:65, 0:64] is the K=1 lhsT that broadcasts the
        # denominator reciprocal from partition 64 to partitions 0..63
        ones_sb = consts.tile([128, 64], bf16, tag="ones_sb")
        nc.vector.memset(ones_sb, 1.0)

        # ---- projections + L-logit production, software-pipelined -----------
        # L(0)/L(1) need only the ic=0 half of qT, so emit proj-q ic0 first,
        # then stream L(0)/L(1) chunks interleaved with the remaining
        # projection work.
        qT_sb = consts.tile([128, 2, N], bf16, tag="qT_sb")
        kT_sb = consts.tile([128, 2, N], bf16, tag="kT_sb")
        v_sb = consts.tile([128, NRC, HPC, 65], bf16, tag="v_sb")
        nc.vector.memset(v_sb[:, :, :, 64], 1.0)

        def emit_qk_unit(dst_sb, w_sb, ic, nw):
            ps = psb.tile([128, 1024], fp32, tag="big")
            for dc in range(4):
                nc.tensor.matmul(
                    ps[:, 0:512],
                    lhsT=w_sb[:, dc, ic * 128:(ic + 1) * 128],
                    rhs=xT_sb[:, dc, nw * 512:(nw + 1) * 512],
                    start=(dc == 0), stop=(dc == 3))
            nc.vector.tensor_copy(dst_sb[:, ic, nw * 512:(nw + 1) * 512],
                                  ps[:, 0:512])

        def emit_v_unit(rc):
            ps = psb.tile([128, 1024], fp32, tag="big")
            for dc in range(4):
                nc.tensor.matmul(
                    ps[:, 0:256],
                    lhsT=xT_sb[:, dc, rc * 128:(rc + 1) * 128],
                    rhs=wv_sb[:, dc, :],
                    start=(dc == 0), stop=(dc == 3))
            nc.vector.tensor_copy(
                v_sb[:, rc, :, 0:64],
                ps[:, 0:256].rearrange("p (h d) -> p h d", h=HPC))

        L_dram_h = [None] * HPC
        for h in range(HPC):
            L_dram_h[h] = dramp.tile([N, W], bf16, tag="Ldram",
                                     name=f"Ldram{h}")

        # L chunks are emitted in pairs sharing one tail PSUM tile
        def emit_L_pair(chunks, cast_on_act=False):
            tail = psb.tile([128, 1024], fp32, tag="big")
            for idx, (h, nck) in enumerate(chunks):
                hc, hp = h // 2, (h % 2) * 64
                lsb = lpool.tile([128, W], bf16, tag="lsb")
                psA = psb.tile([128, 1024], fp32, tag="big")
                for jw in range(2):
                    nc.tensor.matmul(
                        psA[:, jw * 512:(jw + 1) * 512],
                        lhsT=qT_sb[hp:hp + 64, hc, nck * 128:(nck + 1) * 128],
                        rhs=relT_sb[hp:hp + 64, jw * 512:(jw + 1) * 512],
                        start=True, stop=True)
                nc.tensor.matmul(
                    tail[:, idx * 512:idx * 512 + 256],
                    lhsT=qT_sb[hp:hp + 64, hc, nck * 128:(nck + 1) * 128],
                    rhs=relT_sb[hp:hp + 64, 1024:1280],
                    start=True, stop=True)
                if cast_on_act:
                    nc.scalar.copy(lsb[:, 0:1024], psA)
                else:
                    nc.vector.tensor_copy(lsb[:, 0:1024], psA)
                nc.vector.tensor_copy(lsb[:, 1024:1280],
                                      tail[:, idx * 512:idx * 512 + 256])
                nc.gpsimd.dma_start(
                    out=L_dram_h[h][nck * 128:(nck + 1) * 128, :], in_=lsb)

        ep_all_h = [None] * HPC

        def emit_transposes(h):
            """Merged skew transpose-DMAs: one per r-chunk, covering all its
            band n-subchunks (diagonals d = rc - s in [-4, 4])."""
            L_dram = L_dram_h[h]
            ep_all = eppool.tile([128, NRC, 9, 128], bf16, tag="ep_all",
                                 name=f"ep_all{h}")
            ep_all_h[h] = ep_all
            for rc in range(NRC):
                s_lo, s_hi = max(0, rc - 4), min(NRC - 1, rc + 4)
                k = s_hi - s_lo + 1
                # src element (s, a, c) = L[128*s + c, PAD + 128*(rc-s) + 512 + a - c]
                off = (L_dram.offset + 128 * s_lo * W
                       + PAD_L + 128 * (rc - s_lo) + 512)
                src = bass.AP(tensor=L_dram.tensor, offset=off,
                              ap=[[W - 1, 128 * k], [1, 128]])
                slot0 = s_lo - (rc - 4)
                nc.sync.dma_start(out=ep_all[:, rc, slot0:slot0 + k, :],
                                  in_=src, transpose=True)

        def emit_unit(u):
            if u[0] == "qk":
                emit_qk_unit(u[1], u[2], u[3], u[4])
            else:
                emit_v_unit(u[1])

        for nw in range(4):
            emit_qk_unit(qT_sb, wq_sb, 0, nw)
        # startup minimal set: k-ic0 (att(0) dots) + v (att(0) PV) + L(0)
        unit_q = [("qk", kT_sb, wk_sb, 0, nw) for nw in range(4)] \
            + [("v", rc) for rc in range(NRC)]
        ri = 0
        for pr in range(8):
            emit_L_pair([(0, 2 * pr), (0, 2 * pr + 1)], cast_on_act=True)
            for _ in range(2):
                if ri < len(unit_q):
                    emit_unit(unit_q[ri]); ri += 1
        while ri < len(unit_q):
            emit_unit(unit_q[ri]); ri += 1
        emit_transposes(0)

        avn_all = consts.tile([128, 2, NW, 256], bf16, tag="avn_all")

        def emit_att(h, extras=None, next_transposes=None):
            hc, hp = h // 2, (h % 2) * 64
            extras = extras or {}
            num_all = numep.tile([65, NW, 256], bf16, tag="num_all")
            ep_all = ep_all_h[h]

            deferred = []   # emissions delayed into the next window

            for nw in range(NW):
                n0 = nw * 256
                s0 = 2 * nw                     # first n-sub of this window
                P_sb = ppool.tile([128, NRC, 256], bf16, tag="P_sb")
                acc = psa.tile([128, 256], fp32, tag="acc")
                ca, cb = max(0, s0 - 3), min(NRC - 1, s0 + 4)

                def emit_group(g, ep_all=ep_all, n0=n0, s0=s0, ca=ca, cb=cb,
                               P_sb=P_sb):
                    ps = psb.tile([128, 1024], fp32, tag="big")
                    for i in range(4):
                        rc = 4 * g + i
                        nc.tensor.matmul(
                            ps[:, i * 256:(i + 1) * 256],
                            lhsT=kT_sb[hp:hp + 64, hc, rc * 128:(rc + 1) * 128],
                            rhs=qT_sb[hp:hp + 64, hc, n0:n0 + 256],
                            start=True, stop=True)
                    psv = ps.rearrange("p (i n) -> p i n", i=4)
                    # pre-exp band-logit adds restricted to this group's rc's
                    a, b = max(ca, 4 * g), min(cb, 4 * g + 3)
                    if b >= a:
                        kk = b - a + 1
                        epa = ep_all[:, a, s0 - a + 4, 0:128]   # first element
                        ep_run = bass.AP(
                            tensor=epa.tensor, offset=epa.offset,
                            ap=[list(epa.ap[0]), [8 * 128, kk], [1, 256]])
                        nc.vector.tensor_add(psv[:, a - 4 * g:b - 4 * g + 1, :],
                                             psv[:, a - 4 * g:b - 4 * g + 1, :],
                                             ep_run)
                    rc = s0 - 4
                    if 4 * g <= rc <= 4 * g + 3:
                        c0 = (rc - 4 * g) * 256
                        nc.vector.tensor_add(ps[:, c0:c0 + 128],
                                             ps[:, c0:c0 + 128],
                                             ep_all[:, rc, 8, :])
                    rc = s0 + 5
                    if 4 * g <= rc <= 4 * g + 3 and rc <= NRC - 1:
                        c0 = (rc - 4 * g) * 256 + 128
                        nc.vector.tensor_add(ps[:, c0:c0 + 128],
                                             ps[:, c0:c0 + 128],
                                             ep_all[:, rc, 0, :])
                    nc.scalar.activation(
                        P_sb[:, 4 * g:4 * (g + 1), :], psv, EXP,
                        bias=bias_sb)

                def emit_pv(rcs):
                    for rc in rcs:
                        nc.tensor.matmul(
                            acc[0:65, :],
                            lhsT=v_sb[:, rc, h, :],
                            rhs=P_sb[:, rc, :],
                            start=(rc == 0), stop=(rc == NRC - 1),
                            skip_group_check=True)

                emit_group(0)
                emit_group(1)
                emit_group(2)
                emit_group(3)
                if deferred:
                    deferred[0]()
                for th_x in extras.get(nw, ()):
                    th_x()
                if deferred:
                    deferred[1]()
                deferred = []

                def make_tail(P_sb=P_sb, acc=acc, nw=nw, n0=n0, s0=s0):
                    right = list(range(min(s0 + 6, NRC), NRC))
                    band = list(range(0, min(s0 + 6, NRC)))

                    def t1():
                        # right-clamp region first, then scale the partial
                        # accumulator by exp(delta)[n] in one [65,256] op
                        for i, rc in enumerate(right):
                            nc.tensor.matmul(
                                acc[0:65, :],
                                lhsT=v_sb[:, rc, h, :],
                                rhs=P_sb[:, rc, :],
                                start=(i == 0), stop=False,
                                skip_group_check=True)
                        if right:
                            nc.vector.tensor_mul(
                                acc[0:65, :], acc[0:65, :],
                                edel_sb[0:65, h, n0:n0 + 256])
                        rc5 = s0 + 5
                        if rc5 <= NRC - 1:
                            # mixed tile: first half is right-clamp
                            nc.vector.tensor_mul(
                                P_sb[:, rc5, 0:128], P_sb[:, rc5, 0:128],
                                edel_sb[:, h, n0:n0 + 128])

                    def t2():
                        for i, rc in enumerate(band):
                            nc.tensor.matmul(
                                acc[0:65, :],
                                lhsT=v_sb[:, rc, h, :],
                                rhs=P_sb[:, rc, :],
                                start=(not right and i == 0),
                                stop=(rc == band[-1]),
                                skip_group_check=True)
                        nc.vector.tensor_copy(num_all[:, nw, :], acc[0:65, :])
                        # per-window softmax division: 1/den on partition 64,
                        # matmul-broadcast down to partitions 0..63, multiply
                        r_tile = work.tile([128, 256], fp32, tag="r_tile")
                        with nc.allow_low_precision(
                                reason="fp16 1/den; den scaled to ~2^4"):
                            nc.vector.reciprocal(r_tile[64:65, :],
                                                 acc[64:65, :])
                        bc = psa.tile([128, 256], fp32, tag="acc",
                                      name="bc")
                        nc.tensor.matmul(
                            bc[0:64, :],
                            lhsT=ones_sb[64:65, :],
                            rhs=r_tile[64:65, :],
                            start=True, stop=True)
                        if hp == 0:
                            nc.vector.tensor_mul(avn_all[0:64, hc, nw, :],
                                                 num_all[0:64, nw, :],
                                                 bc[0:64, :])
                        else:
                            # odd head lives on partitions 64-127; DVE can't
                            # shift partitions, so hop through a DMA
                            at = work.tile([64, 256], bf16, tag="avn_tmp")
                            nc.vector.tensor_mul(at, num_all[0:64, nw, :],
                                                 bc[0:64, :])
                            nc.gpsimd.dma_start(
                                out=avn_all[64:128, hc, nw, :], in_=at)
                    return [t1, t2]
                deferred = make_tail()

            if deferred:
                deferred[0]()
                deferred[1]()
            if next_transposes is not None:
                emit_transposes(next_transposes)

        def emit_out_unit(nw):
            # output projection for one n-window (head-pair packed matmuls)
            for oc in range(4):
                ps = psb.tile([128, 1024], fp32, tag="big")
                for hcc in range(2):
                    nc.tensor.matmul(
                        ps[:, oc * 256:(oc + 1) * 256],
                        lhsT=wo_sb[:, hcc, oc * 128:(oc + 1) * 128],
                        rhs=avn_all[:, hcc, nw, :],
                        start=(hcc == 0), stop=(hcc == 1))
                o_sb = work.tile([128, 256], fp32, tag="o_sb")
                nc.vector.tensor_copy(o_sb, ps[:, oc * 256:(oc + 1) * 256])
                nc.gpsimd.dma_start(
                    out=outT.ap()[oc * 128:(oc + 1) * 128,
                                  nw * 256:(nw + 1) * 256],
                    in_=o_sb)

        # heads pipeline: startup produced qT/kT-ic0/v/L(0); everything else
        # streams inside the att window loops.
        def lp(h, pr):
            return lambda: emit_L_pair([(h, 2 * pr), (h, 2 * pr + 1)])

        def un(u):
            return lambda: emit_unit(u)

        def tp(h):
            return lambda: emit_transposes(h)

        uq1 = [("qk", qT_sb, wq_sb, 1, nw) for nw in range(4)]
        uk1 = [("qk", kT_sb, wk_sb, 1, nw) for nw in range(4)]
        emit_att(0, extras={
            0: [lp(1, 0), lp(1, 1)],
            1: [lp(1, 2), lp(1, 3)],
            2: [lp(1, 4), lp(1, 5)],
            3: [lp(1, 6), lp(1, 7)],
            4: [tp(1)],
            5: [un(uq1[0]), un(uq1[1])],
            6: [un(uq1[2]), un(uq1[3]), un(uk1[0])],
            7: [un(uk1[1]), un(uk1[2]), un(uk1[3])],
        })
        emit_att(1, extras={
            0: [lp(2, 0), lp(2, 1)], 1: [lp(2, 2), lp(2, 3)],
            2: [lp(2, 4), lp(2, 5)], 3: [lp(2, 6), lp(2, 7)],
            4: [tp(2)],
        })
        emit_att(2, extras={
            0: [lp(3, 0), lp(3, 1)], 1: [lp(3, 2), lp(3, 3)],
            2: [lp(3, 4), lp(3, 5)], 3: [lp(3, 6), lp(3, 7)],
            4: [tp(3)],
        })
        emit_att(3, extras={w: [lambda w=w: emit_out_unit(w - 2)]
                            for w in range(2, NW)})

        # ---- output projection: last two windows ---------------------------
        emit_out_unit(NW - 2)
        emit_out_unit(NW - 1)

    nc.compile()
    return nc


def host_prep(x, Wq, Wkv, Wo, bo, rel_emb):
    """Build the 8 per-core input maps (all host-side prep is O(N*D))."""
    x = np.asarray(x, np.float32)
    Wq = np.asarray(Wq, np.float32)
    Wkv = np.asarray(Wkv, np.float32)
    Wo = np.asarray(Wo, np.float32)
    rel_emb = np.asarray(rel_emb, np.float32)

    # relF[j] = rel_emb[1024-j] - rel_emb[1024], edge-padded/clipped; [W, 64]
    jgrid = np.clip(np.arange(W) - PAD_L, 0, 1024)
    relF = rel_emb[1024 - jgrid] - rel_emb[1024]
    relT_one = np.ascontiguousarray(relF.T)            # [64, W]
    relT_in = np.concatenate([relT_one, relT_one], axis=0).astype(BF)  # [128, W]
    d_vec = rel_emb[0] - rel_emb[1024]                 # [64]

    in_maps = []
    for core in range(NCORES):
        b, hg = core // 2, core % 2
        sl = slice(hg * 256, (hg + 1) * 256)
        wq_s = (Wq[:, sl] * SCALE).astype(BF)
        # edelta per head (LOGITS): x @ Wq~_h @ d_vec
        delta = x[b] @ ((Wq[:, sl] * SCALE).reshape(DIM, HPC, D) @ d_vec)  # [N, HPC]
        in_maps.append({
            "xT": np.ascontiguousarray(x[b].T).astype(BF),
            "wq": wq_s,
            "wk": Wkv[:, sl].astype(BF),
            "wv": Wkv[:, 512 + hg * 256: 512 + (hg + 1) * 256].astype(BF),
            "relT": relT_in,
            "wo": Wo[sl, :].astype(BF),
            "edel": np.ascontiguousarray(np.exp(delta).T).astype(BF),  # [HPC, N]
        })
    return in_maps


def _install_ntff_hook():
    """The agent image's antenv lacks axon_hooks; synthesize it so
    run_bass_kernel_spmd(trace=True) can capture NTFF profiles."""
    import types
    try:
        if "antenv.axon_hooks" not in sys.modules:
            import antenv
            from trn_agent_boot.trn_boot import _ntff_profile_via_ctypes
            hooks = types.ModuleType("antenv.axon_hooks")
            state = {"h": _ntff_profile_via_ctypes("/opt/axon/libaxon_pjrt.so")}
            hooks.set_axon_ntff_profile_hook = lambda h: state.__setitem__("h", h)
            hooks.get_axon_ntff_profile_hook = lambda: state["h"]
            sys.modules["antenv.axon_hooks"] = hooks
            antenv.axon_hooks = hooks
        import antenv.axon_hooks as ah
        return ah.get_axon_ntff_profile_hook() is not None
    except Exception as e:
        print(f"ntff hook install failed: {e!r}")
        return False


def kernel(x, Wq, Wkv, Wo, bo, rel_emb, _trace=False):
    import concourse.bass_utils as bu
    from concourse.bass_utils import run_bass_kernel_spmd

    if "nc" not in _CACHE:
        _CACHE["nc"] = _build_bass()
    nc = _CACHE["nc"]

    in_maps = host_prep(x, Wq, Wkv, Wo, bo, rel_emb)
    kw = {}
    if _trace and _install_ntff_hook():
        bu.upload_artifacts = lambda d: d     # zero-egress: keep artifacts local
        tmpdir = "/root/problem/traces/latest"
        import shutil
        shutil.rmtree(tmpdir, ignore_errors=True)
        os.makedirs(tmpdir, exist_ok=True)
        kw = dict(trace=True, tmpdir=tmpdir)
    res = run_bass_kernel_spmd(nc, in_maps, list(range(NCORES)), **kw)
    _CACHE["last_result"] = res

    bo = np.asarray(bo, np.float32)
    out = np.empty((B, N, DIM), np.float32)
    for b in range(B):
        pT = res.results[2 * b]["outT"] + res.results[2 * b + 1]["outT"]
        out[b] = pT.T + bo[None, :]
    return out
